# revision 72
# baseline (speedup 1.0000x reference)
"""Trainium2 Bass kernel for ConvGlobalLocalCapsuleLayer.

Per-capsule 3x3 SAME conv (8 capsules, 16->128 ch) + 3 iterations of dynamic
routing (softmax over output capsules, squash nonlinearity).

Sharding: data-parallel over batch B=32 across 8 cores (4 images/core),
weights replicated. No cross-core communication.

v4 changes vs v3 (275us -> 257us): engines are in-order queues, so emission
order is the schedule.  Three head-of-line fixes: conv(k+4) moved to the wave
TAIL (it sat in PE's queue between softmax and apply, blocking the expi
matmuls that DVE's rb3 stream waits on, and now instead feeds PE across the
next wave's rv_a gap); the squash split out of r_apply so its ACT-dependent
onecc/ones matmuls emit after both applies' expi/idm; the squash square moved
Pool -> ACT and the w-multiply Pool -> DVE (Pool hops in the squash chain
cost ~1.3us of spine each; removing both: 257 -> 247us).  The it-2 agreement's Pool slice shrunk to 1 capsule (it gated the
ei stop-matmul -> softmax).

v3 changes vs v2 (302.5us -> 275us in the TimelineSim cost model):
  - squash rewritten around a single PSUM-side u = 1+s2 (onecc matmul plus a
    rank-1 ones matmul), an Identity pb-copy that folds bias+scale once, and
    the squares / w-multiply moved to the idle Pool engine.  DVE now only
    does the reciprocal and one fp16 2x multiply per squash.
  - ACT function-table thrash bounded: all Exps of a wave are emitted
    adjacently and all Sqrts likewise (2 table loads per wave instead of ~5).
  - the it-3 routing chain (which feeds this wave's output DMA) is kept
    entirely on DVE and emitted ahead of the slack it-2 chain; only the
    it-2 chain donates slices to the slower Pool engine.
  - conv lookahead deepened to k+4 with votes bufs=6.
"""
import sys

sys.path.insert(0, "/opt/trn_rl_repo")
sys.path.insert(0, "/root/.axon_site/_ro/trn_rl_repo")

import numpy as np
from concourse import bacc, mybir, tile
from concourse.bass_utils import run_bass_kernel_spmd

dt = mybir.dt
AF = mybir.ActivationFunctionType
OP = mybir.AluOpType

N_CORES = 8
EPS = 1e-7
IMG, HH, WW, ICAPS, CIN, C, F = 4, 32, 32, 8, 16, 8, 16
CF = 128
PH, PW = 34, 34
IMGLEN = PH * PW            # 1156
PADLEN = IMG * IMGLEN       # 4624
GUARD = 34
NCHUNK = 8                  # 512-position chunks (16 h-rows each)
TAPS = [(dy, dx) for dy in (-1, 0, 1) for dx in (-1, 0, 1)]

_CACHE = {}


def _host_constants(W, b):
    """All lhsT constant matrices + weight arrangements, built host-side."""
    W = np.asarray(W, np.float32)
    b = np.asarray(b, np.float32)
    cst = {}
    w_s = np.zeros((128, 9 * CF), np.float16)
    for t, (dy, dx) in enumerate(TAPS):
        w_s[:, t * CF:(t + 1) * CF] = W[:, dy + 1, dx + 1, :, :].reshape(128, CF)
    cst["w_s"] = w_s
    w_c = np.zeros((96, ICAPS * 3 * CF), np.float16)
    for i in range(ICAPS):
        q = i % 2
        for dxi in range(3):
            blk = np.zeros((96, CF), np.float16)
            for dyi in range(3):
                blk[q * 48 + dyi * 16:q * 48 + dyi * 16 + 16] = W[i, dyi, dxi]
            w_c[:, (i * 3 + dxi) * CF:(i * 3 + dxi + 1) * CF] = blk
    cst["w_c"] = w_c
    ei = np.zeros((CF, ICAPS * 64), np.float16)
    expi = np.zeros((128, ICAPS * CF), np.float16)
    for i in range(ICAPS):
        for c in range(C):
            for f in range(F):
                ei[c * F + f, i * 64 + i * C + c] = 1.0
                # rows duplicated so odd-chunk (partition-64-based) slices work
                expi[i * C + c, i * CF + c * F + f] = 1.0
                expi[64 + i * C + c, i * CF + c * F + f] = 1.0
    cst["ei"] = ei
    cst["expi"] = expi
    onecc = np.zeros((CF, CF), np.float32)
    for c in range(C):
        onecc[c * F:(c + 1) * F, c * F:(c + 1) * F] = 1.0
    cst["onecc"] = onecc.astype(np.float16)
    # block-diag ones over c within each (half, i) block: Z per (i, n)
    oneii = np.zeros((128, 128), np.float32)
    for h in range(2):
        for i in range(ICAPS):
            b0 = h * 64 + i * C
            oneii[b0:b0 + C, b0:b0 + C] = 1.0
    cst["oneii"] = oneii
    cst["idm16"] = np.eye(128, dtype=np.float16)
    cst["idm32"] = np.eye(128, dtype=np.float32)
    cst["bvec"] = b.reshape(CF, 1)
    cst["b8vec"] = 8.0 * b.reshape(CF, 1)
    cst["epsv"] = np.full((128, 1), EPS, np.float32)
    cst["epsm1"] = np.full((128, 1), EPS - 1.0, np.float32)
    cst["onesc"] = np.ones((1, 128), np.float16)
    cst["onesr"] = np.ones((1, 512), np.float16)
    return cst


_CONST_SPECS = [
    ("w_s", [128, 9 * CF], dt.float16),
    ("w_c", [96, ICAPS * 3 * CF], dt.float16),
    ("ei", [CF, ICAPS * 64], dt.float16),
    ("expi", [128, ICAPS * CF], dt.float16),
    ("onecc", [CF, CF], dt.float16),
    ("oneii", [128, 128], dt.float32r),
    ("idm16", [128, 128], dt.float16),
    ("idm32", [128, 128], dt.float32),
    ("bvec", [CF, 1], dt.float32),
    ("b8vec", [CF, 1], dt.float32),
    ("epsv", [128, 1], dt.float32),
    ("epsm1", [128, 1], dt.float32),
    ("onesc", [1, 128], dt.float16),
    ("onesr", [1, 512], dt.float16),
]


def _build_program():
    nc = bacc.Bacc("TRN2", target_bir_lowering=False, debug=False)
    x_d = nc.dram_tensor("x", [IMG * HH * WW, 128], dt.float32,
                         kind="ExternalInput").ap()
    out_d = nc.dram_tensor("out", [IMG * HH * WW, 128], dt.float16,
                           kind="ExternalOutput").ap()
    cst_d = {n: nc.dram_tensor(n, sh, d, kind="ExternalInput").ap()
             for n, sh, d in _CONST_SPECS}

    with tile.TileContext(nc) as tc:
        with (
            tc.tile_pool(name="const", bufs=1) as cpool,
            tc.tile_pool(name="xbig", bufs=1) as xpool,
            tc.tile_pool(name="xnat", bufs=4) as npool,
            tc.tile_pool(name="work", bufs=2) as wpool,
            tc.tile_pool(name="med", bufs=3) as mpool,
            tc.tile_pool(name="ps_conv", bufs=2, space="PSUM") as ps_conv,
            tc.tile_pool(name="ps_pre", bufs=2, space="PSUM") as ps_pre,
            tc.tile_pool(name="ps_lz", bufs=2, space="PSUM") as ps_lz,
            tc.tile_pool(name="ps_tmp", bufs=2, space="PSUM") as ps_tmp,
        ):
            cst = {}
            for n, sh, d in _CONST_SPECS:
                t = cpool.tile(sh, d, tag=n)
                nc.sync.dma_start(t[:], cst_d[n][:])
                cst[n] = t

            # -------- x load + transpose + cast fp16, split in halves ------
            HLEN = 2 * IMGLEN
            x_sbh = []
            x3h = []

            def xgeom2(ap):
                return ap.rearrange("p (im h w) -> p im h w", im=2, h=PH, w=PW)

            def build_half(half):
                xs = xpool.tile([128, 2 * GUARD + HLEN], dt.float16,
                                tag="x_sb", bufs=2)
                nc.gpsimd.memset(xs[:], 0.0)
                x_sbh.append(xs)
                for quad in range(4):
                    base = (half * 16 + quad * 4) * 128
                    xt = npool.tile([128, 4 * 128], dt.float32, tag="xnat")
                    src4 = x_d[base:base + 512, :].rearrange(
                        "(blk p) c -> p blk c", p=128)
                    nc.sync.dma_start(
                        xt[:].rearrange("p (blk c) -> p blk c", blk=4), src4)
                    for blk in range(4):
                        sub = quad * 4 + blk
                        img_loc, h0 = sub // 8, (sub % 8) * 4
                        tp = ps_conv.tile([128, 128], dt.float32, tag="conv")
                        nc.tensor.transpose(
                            tp[:], xt[:, blk * 128:(blk + 1) * 128],
                            cst["idm32"][:])
                        dst = xgeom2(xs[:, GUARD:GUARD + HLEN])[
                            :, img_loc, h0 + 1:h0 + 5, 1:33]
                        src = tp[:].rearrange("p (h w) -> p h w", h=4, w=WW)
                        nc.scalar.activation(dst, src, AF.Copy)
                x3 = xpool.tile([96, IMG * HLEN], dt.float16, tag="x3", bufs=2)
                for i in range(ICAPS):
                    q, g = i % 2, i // 2
                    for dyi, dy in enumerate((-1, 0, 1)):
                        src = xs[i * 16:(i + 1) * 16,
                                 GUARD + dy * PW:GUARD + dy * PW + HLEN]
                        dst = x3[q * 48 + dyi * 16:q * 48 + dyi * 16 + 16,
                                 g * HLEN:(g + 1) * HLEN]
                        nc.sync.dma_start(dst, src)
                x3h.append(x3)

            build_half(0)

            # ---------------- per-chunk conv + routing --------------------
            st = [dict() for _ in range(NCHUNK)]
            pst = [dict() for _ in range(NCHUNK // 2)]   # per chunk-pair

            def conv_S(ch):
                img, half = ch // 2, ch % 2
                h0 = 1 + 16 * half

                half, img_loc = img // 2, img % 2

                def x3_rhs(i, dx):
                    g = i // 2
                    v = x3h[half][:, g * HLEN:(g + 1) * HLEN]
                    v = v.rearrange("p (im h w) -> p im h w", im=2, h=PH, w=PW)
                    return v[:, img_loc, h0:h0 + 16, 1 + dx:33 + dx]

                def xsb_rhs(dy, dx):
                    v = xgeom2(x_sbh[half][:, GUARD:GUARD + HLEN])
                    return v[:, img_loc, h0 + dy:h0 + dy + 16, 1 + dx:33 + dx]

                votes16 = wpool.tile([128, ICAPS * 512], dt.float16, tag="votes", bufs=6)
                v3 = votes16[:].rearrange("p (i n) -> p i n", i=ICAPS)
                for i in range(ICAPS):
                    vp = ps_conv.tile([128, 512], dt.float32, tag="conv")
                    vps = vp[:].rearrange("p (h w) -> p h w", h=16, w=WW)
                    for dxi, dx in enumerate((-1, 0, 1)):
                        lhsT = cst["w_c"][:, (i * 3 + dxi) * CF:(i * 3 + dxi + 1) * CF]
                        nc.tensor.matmul(vps, lhsT, x3_rhs(i, dx),
                                         start=(dxi == 0), stop=(dxi == 2))
                    # ACT owns the PSUM->SBUF vote copies: DVE is the
                    # bottleneck engine and Pool/GPSIMD cannot access PSUM
                    nc.scalar.activation(v3[:, i, :], vp[:], AF.Copy)

                S = ps_pre.tile([128, 512], dt.float32, tag="pre")
                Ss = S[:].rearrange("p (h w) -> p h w", h=16, w=WW)
                for t, (dy, dx) in enumerate(TAPS):
                    nc.tensor.matmul(Ss, cst["w_s"][:, t * CF:(t + 1) * CF],
                                     xsb_rhs(dy, dx),
                                     start=(t == 0), stop=(t == 8))
                st[ch]["v3"] = v3
                st[ch]["votes16"] = votes16
                st[ch]["S"] = S

            def squash(pre_ps, b_ap, pre_scale, out_dtype, atag):
                """act = pb*w, pb = pre_scale*pre_raw + b,
                w = sqrt(s2+eps)/(1+s2) (==s2/((1+s2)sqrt(s2+eps)) to ~1e-7).

                Single ACT table set {Copy, Ln, Exp, Square}: sqrt done as
                exp(0.5*ln(.)) so no ACT table reloads anywhere.  Squares and
                the w-multiply run on the otherwise-idle Pool engine; DVE only
                does the recip and the final fp16 2x multiply.
                """
                pb = mpool.tile([128, 512], dt.float16, tag="pb")
                nc.scalar.activation(pb[:], pre_ps[:], AF.Identity,
                                     bias=b_ap, scale=pre_scale)
                sq = mpool.tile([128, 512], dt.float16, tag="sq")
                nc.scalar.activation(sq[:], pb[:], AF.Square)
                # u = 1 + s2 accumulated in PSUM (ones-rank-1 matmul adds 1)
                u = ps_tmp.tile([128, 512], dt.float32, tag="tmp")
                nc.tensor.matmul(u[:], cst["onecc"][:], sq[:],
                                 start=True, stop=False)
                nc.tensor.matmul(u[:], cst["onesc"][:], cst["onesr"][:],
                                 start=False, stop=True)
                r = mpool.tile([128, 512], dt.float32, tag="r")
                nc.vector.reciprocal_approx_fast(r[:], u[:])
                st = mpool.tile([128, 512], dt.float16, tag="st")
                nc.scalar.activation(st[:], u[:], AF.Sqrt, bias=cst["epsm1"][:])
                # w-multiply on DVE: a Pool hop here (q7 launch + slow rate)
                # sits on the squash critical chain
                w = mpool.tile([128, 512], dt.float16, tag="w")
                nc.vector.tensor_tensor(w[:], st[:], r[:], OP.mult)
                act = mpool.tile([128, 512], out_dtype, tag=atag)
                nc.vector.tensor_tensor(act[:], pb[:], w[:], OP.mult)
                return act

            def iter1(ch):
                st[ch]["act16"] = squash(st[ch]["S"], cst["bvec"][:], 0.125,
                                         dt.float16, "act1")
                if ch % 2 == 0:
                    lzp = ps_lz.tile([128, 512], dt.float32, tag="lz")
                    pst[ch // 2]["lzp"] = lzp

            def r_agree(ch, it):
                """votes*act multiply + 8 ei matmuls into this chunk's lz half."""
                v3 = st[ch]["v3"]
                act16 = st[ch]["act16"]
                h = ch % 2
                lzp = pst[ch // 2]["lzp"]
                L = lzp[h * 64:h * 64 + 64, :]
                rv_a = wpool.tile([128, ICAPS * 512], dt.float16, tag="rv_a", bufs=3)
                ra3 = rv_a[:].rearrange("p (i n) -> p i n", i=ICAPS)
                if it == 3:
                    # it-3 chain feeds this wave's output: keep it off the
                    # slow Pool engine and split so the first ei matmuls can
                    # start while the second half multiplies.
                    act_b4 = act16[:].unsqueeze(1).broadcast_to([128, 4, 512])
                    nc.vector.tensor_tensor(ra3[:, 0:4, :], v3[:, 0:4, :],
                                            act_b4, OP.mult)
                    nc.vector.tensor_tensor(ra3[:, 4:8, :], v3[:, 4:8, :],
                                            act_b4, OP.mult)
                else:
                    act_b7 = act16[:].unsqueeze(1).broadcast_to([128, 7, 512])
                    act_b1 = act16[:].unsqueeze(1).broadcast_to([128, 1, 512])
                    nc.vector.tensor_tensor(ra3[:, 0:7, :], v3[:, 0:7, :],
                                            act_b7, OP.mult)
                    nc.gpsimd.tensor_tensor(ra3[:, 7:8, :], v3[:, 7:8, :],
                                            act_b1, OP.mult)
                for i in range(ICAPS):
                    nc.tensor.matmul(L, cst["ei"][:, i * 64:(i + 1) * 64],
                                     ra3[:, i, :], start=(it == 2 and i == 0),
                                     stop=(it == 3 and i == ICAPS - 1),
                                     skip_group_check=True)

            def r_soft(ch, it):
                """per-chunk softmax over c (64 partitions, no pair coupling)"""
                h = ch % 2
                lzp = pst[ch // 2]["lzp"]
                L = lzp[h * 64:h * 64 + 64, :]
                ev = mpool.tile([64, 512], dt.float32r, tag="ev")
                nc.scalar.activation(ev[:], L, AF.Exp)
                Z = ps_tmp.tile([64, 512], dt.float32, tag="tmp")
                nc.tensor.matmul(Z[:], cst["oneii"][0:64, 0:64], ev[:],
                                 start=True, stop=True)
                rz = mpool.tile([64, 512], dt.float32, tag="rz")
                nc.vector.reciprocal_approx_fast(rz[:], Z[:])
                route16 = mpool.tile([64, 512], dt.float16, tag="route16")
                if it == 3:
                    nc.vector.tensor_tensor(route16[:],
                                            ev[:].bitcast(dt.float32),
                                            rz[:], OP.mult)
                else:
                    nc.gpsimd.tensor_tensor(route16[:],
                                            ev[:].bitcast(dt.float32),
                                            rz[:], OP.mult)
                st[ch]["route"] = route16

            def r_apply(ch, it):
                """route-weighted vote sum + squash."""
                v3 = st[ch]["v3"]
                route16 = st[ch]["route"]
                rv_b = wpool.tile([128, ICAPS * 512], dt.float16, tag="rv_b")
                rb3 = rv_b[:].rearrange("p (i n) -> p i n", i=ICAPS)
                for i in range(ICAPS):
                    rr = ps_tmp.tile([128, 512], dt.float32, tag="tmp")
                    nc.tensor.matmul(rr[:],
                                     cst["expi"][0:64, i * CF:(i + 1) * CF],
                                     route16[:], start=True, stop=True)
                    nc.vector.tensor_tensor(rb3[:, i, :], v3[:, i, :],
                                            rr[:], OP.mult)
                pre = ps_pre.tile([128, 512], dt.float32, tag="pre")
                for i in range(ICAPS):
                    nc.tensor.matmul(pre[:], cst["idm16"][:], rb3[:, i, :],
                                     start=(i == 0), stop=(i == ICAPS - 1))
                st[ch]["pre"] = pre

            def r_apply_sq(ch, it):
                pre = st[ch].pop("pre")
                if it == 2:
                    st[ch]["act16"] = squash(pre, cst["bvec"][:], 1.0,
                                             dt.float16, "act2")
                else:
                    st[ch]["act_f"] = squash(pre, cst["bvec"][:], 1.0,
                                             dt.float16, "actf")

            def out_chunk(ch):
                act_f = st[ch]["act_f"]
                tp = ps_conv.tile([128, 512], dt.float16, tag="conv")
                for si in range(4):
                    nc.tensor.transpose(tp[:, si * 128:(si + 1) * 128],
                                        act_f[:, si * 128:(si + 1) * 128],
                                        cst["idm16"][:])
                onat = mpool.tile([128, 512], dt.float16, tag="onat")
                nc.scalar.activation(onat[:], tp[:], AF.Copy)
                dst = out_d[ch * 512:(ch + 1) * 512, :].rearrange(
                    "(s p) c -> p s c", p=128)
                nc.sync.dma_start(dst, onat[:].rearrange(
                    "p (s c) -> p s c", s=4))
                st[ch].clear()

            # 3-deep software pipeline: wave k runs R3(k) while R2(k+2)'s
            # chain and conv(k+3) fill the other engines.  Within a wave all
            # softmax Exps are emitted adjacently and all squash Sqrts
            # likewise, so the ACT engine switches function tables at most
            # twice per wave.
            build_half(1)
            conv_S(0)
            iter1(0)
            conv_S(1)
            iter1(1)
            conv_S(2)
            iter1(2)
            conv_S(3)
            iter1(3)
            r_agree(0, 2)
            r_soft(0, 2)
            r_apply(0, 2)
            r_apply_sq(0, 2)
            r_agree(1, 2)
            r_soft(1, 2)
            r_apply(1, 2)
            r_apply_sq(1, 2)
            for k in range(NCHUNK):
                if k + 2 < NCHUNK:
                    r_agree(k + 2, 2)
                r_agree(k, 3)
                if k + 2 < NCHUNK:
                    r_soft(k + 2, 2)
                r_soft(k, 3)
                if k + 2 < NCHUNK:
                    r_apply(k + 2, 2)
                r_apply(k, 3)
                # squash PE matmuls wait on ACT; emitting them after BOTH
                # applies' expi/idm keeps those off PE's head-of-line.
                if k + 2 < NCHUNK:
                    r_apply_sq(k + 2, 2)
                r_apply_sq(k, 3)
                # conv at the wave tail: it would otherwise sit in PE's
                # in-order queue ahead of the applies' expi matmuls (blocking
                # DVE's rb3 stream), and here it feeds PE through the next
                # wave's rv_a gap.
                if k + 4 < NCHUNK:
                    conv_S(k + 4)
                    iter1(k + 4)
                out_chunk(k)

    nc.compile()
    return nc


def kernel(input_tensor, W, b):
    x = np.ascontiguousarray(np.asarray(input_tensor, np.float32))
    B = x.shape[0]
    per = B // N_CORES
    assert x.shape == (32, 32, 32, 8, 16) and per == IMG

    if "nc" not in _CACHE:
        _CACHE["nc"] = _build_program()
    nc = _CACHE["nc"]

    cst = _host_constants(W, b)
    in_maps = []
    for core in range(N_CORES):
        shard = x[core * per:(core + 1) * per].reshape(IMG * HH * WW, 128)
        m = {"x": np.ascontiguousarray(shard)}
        m.update(cst)
        in_maps.append(m)
    res = run_bass_kernel_spmd(nc, in_maps, list(range(N_CORES)))
    out = np.concatenate([res.results[c]["out"].reshape(IMG, HH, WW, C, F)
                          for c in range(N_CORES)], axis=0)
    return out.astype(np.float32)

